# revision 28
# baseline (speedup 1.0000x reference)
"""Trainium2 Bass kernel for nn_CrossAttention_46540265619919.

Cross-attention with gene-axis pre-reduction, causal softmax, residual +
LayerNorm.  Full (unsharded) inputs in, full output out; internally sharded
across 8 NeuronCores as (batch b, row-tile pair): core c -> b = c//2, h = c%2.

Causal-skip schedule: the four 128-row L-tiles of a batch are paired
(wide, narrow) so each core's two slots have uniform score widths
(512, 256) while total causal work stays balanced across the pair:
  h=0 -> tiles {3, 0}  (k-extents 512, 128 -> run as 512, 256)
  h=1 -> tiles {2, 1}  (k-extents 384, 256 -> run as 512, 256)

Schedule (from perfetto analysis of the previous version):
 - One big HWDGE stream on the sync queue carries ck then xq chunks and
   the small transfers.  (The second HWDGE ring (ACT) is NOT used: a NEFF
   that issues DMA from the ACT engine runs every DVE op ~1.2x slower.)
 - The vector engine is the pacing engine (fp32 tensor_tensor = 1 elem/
   cycle).  Emission order keeps it busy: ck trees, slot-0 trees, then
   slot-1 trees with the slot-0 epilogue ops interleaved at points where
   their dependencies (collective -> k_totT -> scores psum) are already
   satisfied, so the in-order queue never head-of-line blocks.
 - Slot-1 streams last with decreasing chunk sizes (128,128,128,64,32,
   16,8,8) so the post-stream tail is only a tiny tree + the epilogue
   chain.
 - The pair k_red exchange is a 64 KB AllGather of pre-transposed
   halves (k-range split), so post-collective work is one readback DMA
   plus the scores matmuls.  (tensor_tensor_reduce wedges the HW; the
   mask-add + max stay as two DVE ops.)
 - Scores matmul operands (qT, k_totT) are float32r: single-pass PE
   matmuls instead of fp32 LOW/HIGH double passes.

Self-contained: hardcodes all shapes; no sibling imports.
"""

import os
from contextlib import ExitStack

import numpy as np

import concourse.bass as bass
import concourse.tile as tile
from concourse import bacc, mybir
from concourse.bass_utils import run_bass_kernel_spmd

F32 = mybir.dt.float32
F32R = mybir.dt.float32r
AX = mybir.AxisListType
OP = mybir.AluOpType
AF = mybir.ActivationFunctionType

# Problem shape (fixed).
B, L, K, GT, GC, D = 4, 512, 512, 512, 256, 64
NCORES = 8
LLOC = L // 2          # 256 L-rows per core (two 128-row slots)
KC = K // 128          # 4 k-chunks of 128
SLOTS = ((512, 4), (256, 2))   # (score width, attn k-blocks) per slot
S0_CHUNKS = (128, 128, 128, 96, 32)
S1_CHUNKS = (128, 128, 128, 64, 32, 16, 8, 8)
K_LOC = K // 2         # each core of a pair reduces full genes for half the k rows
MASK_PENALTY = 1.0e9
LN_EPS = 1e-3
BIG = 3.0e38           # init for the min-reduce that produces -rowmax
USE_TTR = os.environ.get("K_TTR", "0") == "1"
USE_F32R_SCORES = os.environ.get("K_F32R", "1") == "1"
USE_ACT_RING = os.environ.get("K_ACT_RING", "0") == "1"

LAST_RESULTS = None    # BassKernelResults of the most recent run (for test harness)
_CACHED_NC = None


def _ensure_trace_hook():
    """If NTFF tracing is requested but this image's `antenv` lacks
    `axon_hooks`, synthesize it from trn_boot's ctypes path so
    run_bass_kernel_spmd's trace branch doesn't crash. Best-effort."""
    try:
        import antenv.axon_hooks  # noqa: F401
        return
    except ImportError:
        pass
    try:
        import sys
        import types
        import trn_agent_boot.trn_boot as tb
        import concourse.bass_utils as bu
        hook = tb._ntff_profile_via_ctypes("/opt/axon/libaxon_pjrt.so")
        mod = types.ModuleType("antenv.axon_hooks")
        mod.get_axon_ntff_profile_hook = lambda: hook
        mod.set_axon_ntff_profile_hook = lambda h: None
        sys.modules["antenv.axon_hooks"] = mod
        bu.upload_artifacts = lambda tmpdir: tmpdir  # no fish creds in-container
    except Exception:
        os.environ["BASS_NEVER_TRACE"] = "1"  # fall back: run untraced


def _build_program():
    """Build + compile the per-core SPMD Tile program."""
    nc = bacc.Bacc(
        "TRN2",
        target_bir_lowering=False,
        debug=False,
        num_devices=NCORES,
    )

    xq_d = nc.dram_tensor("xq", [LLOC, GT, D], F32, kind="ExternalInput").ap()
    ck_d = nc.dram_tensor("ck", [K_LOC, GC, D], F32, kind="ExternalInput").ap()
    cv_d = nc.dram_tensor("cv", [K, GT], F32, kind="ExternalInput").ap()
    x_d = nc.dram_tensor("xres", [LLOC, GT], F32, kind="ExternalInput").ap()
    thr_d = nc.dram_tensor("thr", [128, 2], F32, kind="ExternalInput").ap()
    out_d = nc.dram_tensor("out", [LLOC, GT], F32, kind="ExternalOutput").ap()

    with tile.TileContext(nc) as tc, ExitStack() as ctx:
        const = ctx.enter_context(tc.tile_pool(name="const", bufs=1))
        stream = ctx.enter_context(tc.tile_pool(name="stream", bufs=4))
        work = ctx.enter_context(tc.tile_pool(name="work", bufs=2))
        smalls = ctx.enter_context(tc.tile_pool(name="smalls", bufs=2))
        ps_mm = ctx.enter_context(tc.tile_pool(name="ps_mm", bufs=3, space="PSUM"))
        ps_tp = ctx.enter_context(tc.tile_pool(name="ps_tp", bufs=2, space="PSUM"))
        dram = ctx.enter_context(tc.tile_pool(name="dram", bufs=1, space="DRAM"))

        def reduce_gene_axis(t, ng, out_ap):
            """Sum t[128, ng, D] over its gene axis into out_ap[128, D]
            on the vector engine: in-place contiguous halving while even
            and >8 gene rows remain, then one short strided reduce."""
            n = ng
            while n > 8 and n % 2 == 0:
                half = n // 2
                nc.vector.tensor_add(t[:, 0:half, :], t[:, 0:half, :], t[:, half:n, :])
                n = half
            nc.vector.tensor_reduce(
                out_ap, t[:, 0:n, :].rearrange("p g d -> p d g"),
                axis=AX.X, op=OP.add,
            )

        # ---- constants (cheap, off the stream path)
        ones = const.tile([128, 128], F32, tag="ones")
        ident = const.tile([128, 128], F32, tag="ident")
        nc.vector.memset(ones[:], 1.0)
        eps_b = const.tile([128, 1], F32, tag="eps_b")
        nc.vector.memset(eps_b[:], LN_EPS)
        nc.gpsimd.affine_select(
            ident[:], ones[:],
            pattern=[[-1, 128]], base=0, channel_multiplier=1,
            compare_op=OP.is_equal, fill=0.0,
        )
        iota_k = const.tile([128, K], F32, tag="iota_k")
        nc.gpsimd.iota(
            iota_k[:], pattern=[[1, K]], base=0, channel_multiplier=0,
            allow_small_or_imprecise_dtypes=True,
        )

        # ---- small transfers (second HWDGE ring if enabled, else sync FIFO)
        sdma = nc.scalar if USE_ACT_RING else nc.sync
        thr_sb = const.tile([128, 2], F32, tag="thr")
        sdma.dma_start(thr_sb[:], thr_d[:, :])
        cv_sb = const.tile([128, KC, GT], F32R, tag="cv")
        cv_stages = []
        for kc in range(KC):
            cv_stage = smalls.tile([128, GT], F32, tag="cv_stage", bufs=4)
            sdma.dma_start(cv_stage[:], cv_d[kc * 128:(kc + 1) * 128, :])
            cv_stages.append(cv_stage)
        for kc in range(KC):
            nc.scalar.copy(cv_sb[:, kc, :], cv_stages[kc][:])
        x_t0 = smalls.tile([128, GT], F32, tag="x_t")
        sdma.dma_start(x_t0[:], x_d[0:128, :])
        x_t1 = smalls.tile([128, GT], F32, tag="x_t")
        sdma.dma_start(x_t1[:], x_d[128:256, :])

        # causal-mask rows (0 / -1e9) built on gpsimd while it is idle
        bits_sb = []
        for s, (W, _) in enumerate(SLOTS):
            bt = const.tile([128, W], F32, tag=f"bits{s}")
            nc.gpsimd.tensor_scalar(
                bt[:], iota_k[:, 0:W], thr_sb[:, s:s + 1], -MASK_PENALTY,
                op0=OP.is_gt, op1=OP.mult,
            )
            bits_sb.append(bt)

        # ---- main HWDGE stream (sync queue): ck chunks first; j0's first
        # gene-half is split in two so the first vector tree starts ~6us
        # earlier (smaller first transfer).
        CK_SPLITS = [(0, 0, 32), (0, 32, 32), (0, 64, 64), (0, 128, 128),
                     (1, 0, 128), (1, 128, 128)]
        ck_tiles = [stream.tile([128, 128, D], F32, name=f"ck_t{i}", tag="stream")
                    for i in range(len(CK_SPLITS))]
        for i, (j, g0, ng) in enumerate(CK_SPLITS):
            nc.sync.dma_start(
                ck_tiles[i][:, 0:ng, :],
                ck_d[j * 128:(j + 1) * 128, g0:g0 + ng, :],
            )

        # ---- xq stream DMAs: s0c0..c3, s1c0, s0c4, s1c1..c7
        NCH0, NCH1 = len(S0_CHUNKS), len(S1_CHUNKS)
        G0_0 = [sum(S0_CHUNKS[:i]) for i in range(NCH0)]
        G0_1 = [sum(S1_CHUNKS[:i]) for i in range(NCH1)]
        W0, NKC0 = SLOTS[0]
        W1, NKC1 = SLOTS[1]

        xq_order = [(0, 0), (0, 1), (1, 5), (0, 2), (1, 6), (0, 3), (1, 0),
                    (0, 4), (1, 1), (1, 2), (1, 3), (1, 4), (1, 7)]
        xq_tiles = {}
        for s, c in xq_order:
            g0 = (G0_0 if s == 0 else G0_1)[c]
            ng = (S0_CHUNKS if s == 0 else S1_CHUNKS)[c]
            t = stream.tile([128, 128, D], F32, name=f"xq_{s}_{c}", tag="stream")
            nc.sync.dma_start(t[:, 0:ng, :], xq_d[s * 128:(s + 1) * 128, g0:g0 + ng, :])
            xq_tiles[(s, c)] = t

        # ---- ck gene trees (vector): per local k-chunk j, sum all 256
        # genes (per-tile partial trees + one tiny add, so each tree
        # starts the moment its 4 MB tile lands).  The kred halves are
        # TRANSPOSED BEFORE the exchange, so the 64 KB AllGather payload
        # is already k_tot^T: after the collective only one readback DMA
        # and the scores matmuls remain on the critical path.
        KTD = F32R if USE_F32R_SCORES else F32
        kredT_in = dram.tile([D, 2, 128], KTD, tag="kredT_in")
        kredT_out = dram.tile([2, D, 256], KTD, tag="kredT_out")
        tile_of_j = {0: (0, 1, 2, 3), 1: (4, 5)}
        for j in range(2):
            parts = []
            for i in tile_of_j[j]:
                ng = CK_SPLITS[i][2]
                kp = smalls.tile([128, D], F32, name=f"kp_{j}_{i}",
                                 tag="k_red", bufs=6)
                reduce_gene_axis(ck_tiles[i][:, 0:ng, :], ng, kp[:])
                parts.append(kp)
            for p in parts[1:]:
                nc.vector.tensor_add(parts[0][:], parts[0][:], p[:])
            tp = ps_tp.tile([D, 128], F32, name=f"ktp{j}", tag="tpose")
            nc.tensor.transpose(tp[:], parts[0][:], ident[:])
            ktj = smalls.tile([D, 128], KTD, name=f"ktj{j}", tag="ktj")
            nc.scalar.copy(ktj[:], tp[:])
            nc.gpsimd.dma_start(kredT_in[:, j, :], ktj[:])
        nc.gpsimd.collective_compute(
            "AllGather", OP.bypass,
            replica_groups=[[2 * b, 2 * b + 1] for b in range(B)],
            ins=[kredT_in.opt()], outs=[kredT_out.opt()],
        )
        # Gathered layout is rank-major flat: rank r's [64, 256] block is
        # exactly k_tot^T columns [256r, 256r+256) -> one unscrambling DMA.
        k_totT = const.tile([64, K], KTD, tag="k_totT")
        for r in range(2):
            nc.gpsimd.dma_start(k_totT[:, 256 * r:256 * (r + 1)], kredT_out[r])

        pending_mms = []
        mm_counts = {}

        def emit_tree(s, c, ps_s, W, nch, mm_inline=True):
            """Vector tree for one xq gene chunk + PE transpose + qT copy.
            The scores matmul is emitted inline (late chunks, k_totT
            already resident) or deferred into one batch (early chunks)
            so the in-order PE queue never blocks on the collective."""
            ng = (S0_CHUNKS if s == 0 else S1_CHUNKS)[c]
            t = xq_tiles[(s, c)]
            qp = smalls.tile([128, D], F32, tag="qp", bufs=8)
            reduce_gene_axis(t[:, 0:ng, :], ng, qp[:])
            tq = ps_tp.tile([D, 128], F32, tag="tpose_q", bufs=3)
            nc.tensor.transpose(tq[:], qp[:], ident[:])
            qT = smalls.tile([D, 128], F32R if USE_F32R_SCORES else F32,
                             tag="qT", bufs=10)
            nc.scalar.copy(qT[:], tq[:])
            nprev = mm_counts.get(id(ps_s), 0)
            mm_counts[id(ps_s)] = nprev + 1
            mm = (ps_s, qT, W, nprev == 0, nprev == nch - 1)
            if mm_inline:
                _emit_mm(mm)
            else:
                pending_mms.append(mm)

        def _emit_mm(mm):
            ps_s, qT, W, start, stop = mm
            nc.tensor.matmul(ps_s[:, 0:W], qT[:], k_totT[:, 0:W],
                             start=start, stop=stop)

        def flush_mms():
            for mm in pending_mms:
                _emit_mm(mm)
            pending_mms.clear()

        ps_s0 = ps_mm.tile([128, K], F32, tag="mm")
        ps_s1 = ps_mm.tile([128, K], F32, tag="mm")

        # vector program follows stream arrival order; scores matmuls of the
        # early chunks are deferred until the whole batch's qTs exist.
        emit_tree(0, 0, ps_s0, W0, NCH0, mm_inline=False)
        emit_tree(0, 1, ps_s0, W0, NCH0, mm_inline=False)
        emit_tree(1, 5, ps_s1, W1, NCH1, mm_inline=False)
        emit_tree(0, 2, ps_s0, W0, NCH0, mm_inline=False)
        emit_tree(1, 6, ps_s1, W1, NCH1, mm_inline=False)
        emit_tree(0, 3, ps_s0, W0, NCH0, mm_inline=False)
        emit_tree(1, 0, ps_s1, W1, NCH1, mm_inline=False)
        emit_tree(0, 4, ps_s0, W0, NCH0, mm_inline=False)

        # ---- epilogue pieces (emitted interleaved with slot-1 trees)
        def epi_scores(s, W, ps_s, bits):
            """masked scores -> (negated) masked + -rowmax + exp row."""
            s_neg = work.tile([128, K], F32, tag="s_neg")
            negmax = smalls.tile([128, 1], F32, tag="negmax")
            if USE_TTR:
                # s_neg = -(scores + bits); negmax = min(s_neg) = -max(masked)
                nc.vector.tensor_tensor_reduce(
                    out=s_neg[:, 0:W], in0=ps_s[:, 0:W], in1=bits[:],
                    scale=-1.0, scalar=BIG, op0=OP.add, op1=OP.min,
                    accum_out=negmax[:],
                )
                exp_scale = -1.0
            else:
                nc.vector.scalar_tensor_tensor(
                    s_neg[:, 0:W], bits[:], 1.0, ps_s[:, 0:W],
                    op0=OP.mult, op1=OP.add,
                )
                nc.vector.tensor_reduce(
                    negmax[:], s_neg[:, 0:W], axis=AX.X, op=OP.max, negate=True
                )
                exp_scale = 1.0
            w = work.tile([128, K], F32, tag="w")
            denom = smalls.tile([128, 1], F32, tag="denom")
            # w = exp(exp_scale*s_neg + negmax) = exp(masked - rowmax); denom = row sum
            nc.scalar.activation(
                w[:, 0:W], s_neg[:, 0:W], AF.Exp, bias=negmax[:], scale=exp_scale,
                accum_out=denom[:],
            )
            return w, denom

        def epi_attn(w, nkc):
            """w^T per live k-block and attention matmul into PSUM."""
            wT = work.tile([128, KC, 128], F32R, tag="wT")
            for kc in range(nkc):
                tw = ps_tp.tile([128, 128], F32, tag="tpose")
                nc.tensor.transpose(tw[:], w[:, kc * 128:(kc + 1) * 128], ident[:])
                nc.scalar.copy(wT[:, kc, :], tw[:])
            ps_a = ps_mm.tile([128, GT], F32, tag="mm")
            for kc in range(nkc):
                nc.tensor.matmul(
                    ps_a[:], wT[:, kc, :], cv_sb[:, kc, :],
                    start=(kc == 0), stop=(kc == nkc - 1),
                )
            return ps_a

        def epi_norm_a(ps_a, denom, x_t):
            """y = attn/denom + x; batch-norm stats; sqrt on scalar."""
            recip = smalls.tile([128, 1], F32, tag="recip")
            nc.vector.reciprocal(recip[:], denom[:])
            y = work.tile([128, GT], F32, tag="y")
            nc.vector.scalar_tensor_tensor(
                y[:], ps_a[:], recip[:], x_t[:], op0=OP.mult, op1=OP.add
            )
            stats = smalls.tile([128, 6], F32, tag="stats")
            nc.vector.bn_stats(stats[:], y[:])
            mv = smalls.tile([128, 2], F32, tag="mv")
            nc.vector.bn_aggr(mv[:], stats[:])
            std = smalls.tile([128, 1], F32, tag="std")
            nc.scalar.activation(std[:], mv[:, 1:2], AF.Sqrt, bias=eps_b[:], scale=1.0)
            return y, mv, std

        def epi_norm_b(y, mv, std):
            rstd = smalls.tile([128, 1], F32, tag="rstd")
            nc.vector.reciprocal(rstd[:], std[:])
            o_t = work.tile([128, GT], F32, tag="o_t")
            nc.vector.tensor_scalar(
                o_t[:], y[:], mv[:, 0:1], rstd[:], op0=OP.subtract, op1=OP.mult
            )
            return o_t

        # slot-0 epilogue interleaved with slot-1 trees so the in-order
        # vector queue never blocks on a not-yet-satisfied dependency:
        #   ttr needs ps_s0 (ready right after s0c4's matmul)
        #   recip/y/bn need exp0 + attn psum (ready during s1c1's tree)
        #   rstd/TS need sqrt0 (ready during s1c2's tree)
        emit_tree(1, 1, ps_s1, W1, NCH1, mm_inline=False)
        flush_mms()
        for c in (2, 3, 4):
            emit_tree(1, c, ps_s1, W1, NCH1)
        w0, denom0 = epi_scores(0, W0, ps_s0, bits_sb[0])
        ps_a0 = epi_attn(w0, NKC0)
        emit_tree(1, 7, ps_s1, W1, NCH1)
        y0, mv0, std0 = epi_norm_a(ps_a0, denom0, x_t0)
        o_t0 = epi_norm_b(y0, mv0, std0)
        sdma.dma_start(out_d[0:128, :], o_t0[:])

        # slot-1 epilogue (the tail)
        w1, denom1 = epi_scores(1, W1, ps_s1, bits_sb[1])
        ps_a1 = epi_attn(w1, NKC1)
        y1, mv1, std1 = epi_norm_a(ps_a1, denom1, x_t1)
        o_t1 = epi_norm_b(y1, mv1, std1)
        sdma.dma_start(out_d[128:256, :], o_t1[:])

    nc.compile()
    return nc


def _get_nc():
    global _CACHED_NC
    if _CACHED_NC is None:
        _CACHED_NC = _build_program()
    return _CACHED_NC


def _tiles_for(h: int) -> tuple[int, int]:
    """Row-tile indices (slot0, slot1) handled by pair-half h."""
    return 3 - h, h


def kernel(x, x_query, context_key, context_value, gamma, beta):
    global LAST_RESULTS
    x = np.asarray(x, np.float32)
    x_query = np.asarray(x_query, np.float32)
    context_key = np.asarray(context_key, np.float32)
    context_value = np.asarray(context_value, np.float32)
    gamma = np.asarray(gamma, np.float32)
    beta = np.asarray(beta, np.float32)

    nc = _get_nc()
    p_idx = np.arange(128, dtype=np.float32)
    in_maps = []
    for c in range(NCORES):
        b, h = c // 2, c % 2
        t0, t1 = _tiles_for(h)
        r0 = slice(t0 * 128, (t0 + 1) * 128)
        r1 = slice(t1 * 128, (t1 + 1) * 128)
        thr = np.empty((128, 2), np.float32)
        thr[:, 0] = t0 * 128 + p_idx
        thr[:, 1] = t1 * 128 + p_idx
        in_maps.append({
            "xq": np.concatenate([x_query[b, r0], x_query[b, r1]]),
            "ck": np.ascontiguousarray(context_key[b, h * K_LOC:(h + 1) * K_LOC]),
            "cv": np.ascontiguousarray(context_value[b]),
            "xres": np.concatenate([x[b, r0], x[b, r1]]),
            "thr": thr,
        })

    if os.environ.get("KERNEL_TRACE") or os.environ.get("BASS_TRACE"):
        _ensure_trace_hook()
    res = run_bass_kernel_spmd(
        nc,
        in_maps,
        core_ids=list(range(NCORES)),
        trace=bool(os.environ.get("KERNEL_TRACE")),
    )
    LAST_RESULTS = res

    out = np.empty((B, L, GT), np.float32)
    for c, r in enumerate(res.results):
        b, h = c // 2, c % 2
        t0, t1 = _tiles_for(h)
        out[b, t0 * 128:(t0 + 1) * 128] = r["out"][0:128]
        out[b, t1 * 128:(t1 + 1) * 128] = r["out"][128:256]
    # LN affine (gamma/beta broadcast over the last axis) applied on host.
    out = out * gamma + beta
    return out.astype(np.float32)


# revision 29
# speedup vs baseline: 1.0180x; 1.0180x over previous
"""Trainium2 Bass kernel for nn_CrossAttention_46540265619919.

Cross-attention with gene-axis pre-reduction, causal softmax, residual +
LayerNorm.  Full (unsharded) inputs in, full output out; internally sharded
across 8 NeuronCores as (batch b, row-tile pair): core c -> b = c//2, h = c%2.

Causal-skip schedule: the four 128-row L-tiles of a batch are paired
(wide, narrow) so each core's two slots have uniform score widths
(512, 256) while total causal work stays balanced across the pair:
  h=0 -> tiles {3, 0}  (k-extents 512, 128 -> run as 512, 256)
  h=1 -> tiles {2, 1}  (k-extents 384, 256 -> run as 512, 256)

Schedule (from perfetto analysis of the previous version):
 - One big HWDGE stream on the sync queue carries ck then xq chunks and
   the small transfers.  (The second HWDGE ring (ACT) is NOT used: a NEFF
   that issues DMA from the ACT engine runs every DVE op ~1.2x slower.)
 - The vector engine is the pacing engine (fp32 tensor_tensor = 1 elem/
   cycle).  Emission order keeps it busy: ck trees, slot-0 trees, then
   slot-1 trees with the slot-0 epilogue ops interleaved at points where
   their dependencies (collective -> k_totT -> scores psum) are already
   satisfied, so the in-order queue never head-of-line blocks.
 - Slot-1 streams last with decreasing chunk sizes (128,128,128,64,32,
   16,8,8) so the post-stream tail is only a tiny tree + the epilogue
   chain.
 - The pair k_red exchange is a 64 KB AllGather of pre-transposed
   halves (k-range split), so post-collective work is one readback DMA
   plus the scores matmuls.  (tensor_tensor_reduce wedges the HW; the
   mask-add + max stay as two DVE ops.)
 - Scores matmul operands (qT, k_totT) are float32r: single-pass PE
   matmuls instead of fp32 LOW/HIGH double passes.

Self-contained: hardcodes all shapes; no sibling imports.
"""

import os
from contextlib import ExitStack

import numpy as np

import concourse.bass as bass
import concourse.tile as tile
from concourse import bacc, mybir
from concourse.bass_utils import run_bass_kernel_spmd

F32 = mybir.dt.float32
F32R = mybir.dt.float32r
AX = mybir.AxisListType
OP = mybir.AluOpType
AF = mybir.ActivationFunctionType

# Problem shape (fixed).
B, L, K, GT, GC, D = 4, 512, 512, 512, 256, 64
NCORES = 8
LLOC = L // 2          # 256 L-rows per core (two 128-row slots)
KC = K // 128          # 4 k-chunks of 128
SLOTS = ((512, 4), (256, 2))   # (score width, attn k-blocks) per slot
S0_CHUNKS = (128, 128, 128, 96, 32)
S1_CHUNKS = (128, 128, 128, 64, 32, 16, 8, 8)
K_LOC = K // 2         # each core of a pair reduces full genes for half the k rows
MASK_PENALTY = 1.0e9
LN_EPS = 1e-3
BIG = 3.0e38           # init for the min-reduce that produces -rowmax
USE_TTR = os.environ.get("K_TTR", "0") == "1"
USE_F32R_SCORES = os.environ.get("K_F32R", "1") == "1"
USE_ACT_RING = os.environ.get("K_ACT_RING", "0") == "1"

LAST_RESULTS = None    # BassKernelResults of the most recent run (for test harness)
_CACHED_NC = None


def _ensure_trace_hook():
    """If NTFF tracing is requested but this image's `antenv` lacks
    `axon_hooks`, synthesize it from trn_boot's ctypes path so
    run_bass_kernel_spmd's trace branch doesn't crash. Best-effort."""
    try:
        import antenv.axon_hooks  # noqa: F401
        return
    except ImportError:
        pass
    try:
        import sys
        import types
        import trn_agent_boot.trn_boot as tb
        import concourse.bass_utils as bu
        hook = tb._ntff_profile_via_ctypes("/opt/axon/libaxon_pjrt.so")
        mod = types.ModuleType("antenv.axon_hooks")
        mod.get_axon_ntff_profile_hook = lambda: hook
        mod.set_axon_ntff_profile_hook = lambda h: None
        sys.modules["antenv.axon_hooks"] = mod
        bu.upload_artifacts = lambda tmpdir: tmpdir  # no fish creds in-container
    except Exception:
        os.environ["BASS_NEVER_TRACE"] = "1"  # fall back: run untraced


def _build_program():
    """Build + compile the per-core SPMD Tile program."""
    nc = bacc.Bacc(
        "TRN2",
        target_bir_lowering=False,
        debug=False,
        num_devices=NCORES,
    )

    xq_d = nc.dram_tensor("xq", [LLOC, GT, D], F32, kind="ExternalInput").ap()
    ck_d = nc.dram_tensor("ck", [K_LOC, GC, D], F32, kind="ExternalInput").ap()
    cv_d = nc.dram_tensor("cv", [K, GT], F32, kind="ExternalInput").ap()
    x_d = nc.dram_tensor("xres", [LLOC, GT], F32, kind="ExternalInput").ap()
    thr_d = nc.dram_tensor("thr", [128, 2], F32, kind="ExternalInput").ap()
    out_d = nc.dram_tensor("out", [LLOC, GT], F32, kind="ExternalOutput").ap()

    with tile.TileContext(nc) as tc, ExitStack() as ctx:
        const = ctx.enter_context(tc.tile_pool(name="const", bufs=1))
        stream = ctx.enter_context(tc.tile_pool(name="stream", bufs=4))
        work = ctx.enter_context(tc.tile_pool(name="work", bufs=2))
        smalls = ctx.enter_context(tc.tile_pool(name="smalls", bufs=2))
        ps_mm = ctx.enter_context(tc.tile_pool(name="ps_mm", bufs=3, space="PSUM"))
        ps_tp = ctx.enter_context(tc.tile_pool(name="ps_tp", bufs=2, space="PSUM"))
        dram = ctx.enter_context(tc.tile_pool(name="dram", bufs=1, space="DRAM"))

        def reduce_gene_axis(t, ng, out_ap):
            """Sum t[128, ng, D] over its gene axis into out_ap[128, D]
            on the vector engine: in-place contiguous halving while even
            and >8 gene rows remain, then one short strided reduce."""
            n = ng
            while n > 8 and n % 2 == 0:
                half = n // 2
                nc.vector.tensor_add(t[:, 0:half, :], t[:, 0:half, :], t[:, half:n, :])
                n = half
            nc.vector.tensor_reduce(
                out_ap, t[:, 0:n, :].rearrange("p g d -> p d g"),
                axis=AX.X, op=OP.add,
            )

        # ---- constants (cheap, off the stream path)
        ones = const.tile([128, 128], F32, tag="ones")
        ident = const.tile([128, 128], F32, tag="ident")
        nc.vector.memset(ones[:], 1.0)
        eps_b = const.tile([128, 1], F32, tag="eps_b")
        nc.vector.memset(eps_b[:], LN_EPS)
        nc.gpsimd.affine_select(
            ident[:], ones[:],
            pattern=[[-1, 128]], base=0, channel_multiplier=1,
            compare_op=OP.is_equal, fill=0.0,
        )
        iota_k = const.tile([128, K], F32, tag="iota_k")
        nc.gpsimd.iota(
            iota_k[:], pattern=[[1, K]], base=0, channel_multiplier=0,
            allow_small_or_imprecise_dtypes=True,
        )

        # ---- small transfers (second HWDGE ring if enabled, else sync FIFO)
        sdma = nc.scalar if USE_ACT_RING else nc.sync
        thr_sb = const.tile([128, 2], F32, tag="thr")
        sdma.dma_start(thr_sb[:], thr_d[:, :])

        # causal-mask rows (0 / -1e9) built on gpsimd while it is idle
        bits_sb = []
        for s, (W, _) in enumerate(SLOTS):
            bt = const.tile([128, W], F32, tag=f"bits{s}")
            nc.gpsimd.tensor_scalar(
                bt[:], iota_k[:, 0:W], thr_sb[:, s:s + 1], -MASK_PENALTY,
                op0=OP.is_gt, op1=OP.mult,
            )
            bits_sb.append(bt)

        # ---- main HWDGE stream (sync queue): ck chunks first; j0's first
        # gene-half is split in two so the first vector tree starts ~6us
        # earlier (smaller first transfer).
        CK_SPLITS = [(0, 0, 32), (0, 32, 32), (0, 64, 64), (0, 128, 128),
                     (1, 0, 128), (1, 128, 128)]
        ck_tiles = [stream.tile([128, 128, D], F32, name=f"ck_t{i}", tag="stream")
                    for i in range(len(CK_SPLITS))]
        for i, (j, g0, ng) in enumerate(CK_SPLITS):
            nc.sync.dma_start(
                ck_tiles[i][:, 0:ng, :],
                ck_d[j * 128:(j + 1) * 128, g0:g0 + ng, :],
            )

        # cv / residual rows queue AFTER ck (they are consumed only by the
        # epilogues ~150us in; in front of ck they delayed the first tree
        # and the collective by ~4us).
        cv_sb = const.tile([128, KC, GT], F32R, tag="cv")
        cv_stages = []
        for kc in range(KC):
            cv_stage = smalls.tile([128, GT], F32, tag="cv_stage", bufs=4)
            sdma.dma_start(cv_stage[:], cv_d[kc * 128:(kc + 1) * 128, :])
            cv_stages.append(cv_stage)
        for kc in range(KC):
            nc.scalar.copy(cv_sb[:, kc, :], cv_stages[kc][:])
        x_t0 = smalls.tile([128, GT], F32, tag="x_t")
        sdma.dma_start(x_t0[:], x_d[0:128, :])
        x_t1 = smalls.tile([128, GT], F32, tag="x_t")
        sdma.dma_start(x_t1[:], x_d[128:256, :])

        # ---- xq stream DMAs: s0c0..c3, s1c0, s0c4, s1c1..c7
        NCH0, NCH1 = len(S0_CHUNKS), len(S1_CHUNKS)
        G0_0 = [sum(S0_CHUNKS[:i]) for i in range(NCH0)]
        G0_1 = [sum(S1_CHUNKS[:i]) for i in range(NCH1)]
        W0, NKC0 = SLOTS[0]
        W1, NKC1 = SLOTS[1]

        xq_order = [(0, 0), (0, 1), (1, 5), (0, 2), (1, 6), (0, 3), (1, 0),
                    (0, 4), (1, 1), (1, 2), (1, 3), (1, 4), (1, 7)]
        xq_tiles = {}
        for s, c in xq_order:
            g0 = (G0_0 if s == 0 else G0_1)[c]
            ng = (S0_CHUNKS if s == 0 else S1_CHUNKS)[c]
            t = stream.tile([128, 128, D], F32, name=f"xq_{s}_{c}", tag="stream")
            nc.sync.dma_start(t[:, 0:ng, :], xq_d[s * 128:(s + 1) * 128, g0:g0 + ng, :])
            xq_tiles[(s, c)] = t

        # ---- ck gene trees (vector): per local k-chunk j, sum all 256
        # genes (per-tile partial trees + one tiny add, so each tree
        # starts the moment its 4 MB tile lands).  The kred halves are
        # TRANSPOSED BEFORE the exchange, so the 64 KB AllGather payload
        # is already k_tot^T: after the collective only one readback DMA
        # and the scores matmuls remain on the critical path.
        KTD = F32R if USE_F32R_SCORES else F32
        kredT_in = dram.tile([D, 2, 128], KTD, tag="kredT_in")
        kredT_out = dram.tile([2, D, 256], KTD, tag="kredT_out")
        tile_of_j = {0: (0, 1, 2, 3), 1: (4, 5)}
        for j in range(2):
            parts = []
            for i in tile_of_j[j]:
                ng = CK_SPLITS[i][2]
                kp = smalls.tile([128, D], F32, name=f"kp_{j}_{i}",
                                 tag="k_red", bufs=6)
                reduce_gene_axis(ck_tiles[i][:, 0:ng, :], ng, kp[:])
                parts.append(kp)
            for p in parts[1:]:
                nc.vector.tensor_add(parts[0][:], parts[0][:], p[:])
            tp = ps_tp.tile([D, 128], F32, name=f"ktp{j}", tag="tpose")
            nc.tensor.transpose(tp[:], parts[0][:], ident[:])
            ktj = smalls.tile([D, 128], KTD, name=f"ktj{j}", tag="ktj")
            nc.scalar.copy(ktj[:], tp[:])
            nc.gpsimd.dma_start(kredT_in[:, j, :], ktj[:])
        nc.gpsimd.collective_compute(
            "AllGather", OP.bypass,
            replica_groups=[[2 * b, 2 * b + 1] for b in range(B)],
            ins=[kredT_in.opt()], outs=[kredT_out.opt()],
        )
        # Gathered layout is rank-major flat: rank r's [64, 256] block is
        # exactly k_tot^T columns [256r, 256r+256) -> one unscrambling DMA.
        k_totT = const.tile([64, K], KTD, tag="k_totT")
        for r in range(2):
            nc.gpsimd.dma_start(k_totT[:, 256 * r:256 * (r + 1)], kredT_out[r])

        pending_mms = []
        mm_counts = {}

        def emit_tree(s, c, ps_s, W, nch, mm_inline=True):
            """Vector tree for one xq gene chunk + PE transpose + qT copy.
            The scores matmul is emitted inline (late chunks, k_totT
            already resident) or deferred into one batch (early chunks)
            so the in-order PE queue never blocks on the collective."""
            ng = (S0_CHUNKS if s == 0 else S1_CHUNKS)[c]
            t = xq_tiles[(s, c)]
            qp = smalls.tile([128, D], F32, tag="qp", bufs=8)
            reduce_gene_axis(t[:, 0:ng, :], ng, qp[:])
            tq = ps_tp.tile([D, 128], F32, tag="tpose_q", bufs=3)
            nc.tensor.transpose(tq[:], qp[:], ident[:])
            qT = smalls.tile([D, 128], F32R if USE_F32R_SCORES else F32,
                             tag="qT", bufs=10)
            nc.scalar.copy(qT[:], tq[:])
            nprev = mm_counts.get(id(ps_s), 0)
            mm_counts[id(ps_s)] = nprev + 1
            mm = (ps_s, qT, W, nprev == 0, nprev == nch - 1)
            if mm_inline:
                _emit_mm(mm)
            else:
                pending_mms.append(mm)

        def _emit_mm(mm):
            ps_s, qT, W, start, stop = mm
            nc.tensor.matmul(ps_s[:, 0:W], qT[:], k_totT[:, 0:W],
                             start=start, stop=stop)

        def flush_mms():
            for mm in pending_mms:
                _emit_mm(mm)
            pending_mms.clear()

        ps_s0 = ps_mm.tile([128, K], F32, tag="mm")
        ps_s1 = ps_mm.tile([128, K], F32, tag="mm")

        # vector program follows stream arrival order; scores matmuls of the
        # early chunks are deferred until the whole batch's qTs exist.
        emit_tree(0, 0, ps_s0, W0, NCH0, mm_inline=False)
        emit_tree(0, 1, ps_s0, W0, NCH0, mm_inline=False)
        emit_tree(1, 5, ps_s1, W1, NCH1, mm_inline=False)
        emit_tree(0, 2, ps_s0, W0, NCH0, mm_inline=False)
        emit_tree(1, 6, ps_s1, W1, NCH1, mm_inline=False)
        emit_tree(0, 3, ps_s0, W0, NCH0, mm_inline=False)
        emit_tree(1, 0, ps_s1, W1, NCH1, mm_inline=False)
        emit_tree(0, 4, ps_s0, W0, NCH0, mm_inline=False)

        # ---- epilogue pieces (emitted interleaved with slot-1 trees)
        def epi_scores(s, W, ps_s, bits):
            """masked scores -> (negated) masked + -rowmax + exp row."""
            s_neg = work.tile([128, K], F32, tag="s_neg")
            negmax = smalls.tile([128, 1], F32, tag="negmax")
            if USE_TTR:
                # s_neg = -(scores + bits); negmax = min(s_neg) = -max(masked)
                nc.vector.tensor_tensor_reduce(
                    out=s_neg[:, 0:W], in0=ps_s[:, 0:W], in1=bits[:],
                    scale=-1.0, scalar=BIG, op0=OP.add, op1=OP.min,
                    accum_out=negmax[:],
                )
                exp_scale = -1.0
            else:
                nc.vector.scalar_tensor_tensor(
                    s_neg[:, 0:W], bits[:], 1.0, ps_s[:, 0:W],
                    op0=OP.mult, op1=OP.add,
                )
                nc.vector.tensor_reduce(
                    negmax[:], s_neg[:, 0:W], axis=AX.X, op=OP.max, negate=True
                )
                exp_scale = 1.0
            w = work.tile([128, K], F32, tag="w")
            denom = smalls.tile([128, 1], F32, tag="denom")
            # w = exp(exp_scale*s_neg + negmax) = exp(masked - rowmax); denom = row sum
            nc.scalar.activation(
                w[:, 0:W], s_neg[:, 0:W], AF.Exp, bias=negmax[:], scale=exp_scale,
                accum_out=denom[:],
            )
            return w, denom

        def epi_attn(w, nkc):
            """w^T per live k-block and attention matmul into PSUM."""
            wT = work.tile([128, KC, 128], F32R, tag="wT")
            for kc in range(nkc):
                tw = ps_tp.tile([128, 128], F32, tag="tpose")
                nc.tensor.transpose(tw[:], w[:, kc * 128:(kc + 1) * 128], ident[:])
                nc.scalar.copy(wT[:, kc, :], tw[:])
            ps_a = ps_mm.tile([128, GT], F32, tag="mm")
            for kc in range(nkc):
                nc.tensor.matmul(
                    ps_a[:], wT[:, kc, :], cv_sb[:, kc, :],
                    start=(kc == 0), stop=(kc == nkc - 1),
                )
            return ps_a

        def epi_norm_a(ps_a, denom, x_t):
            """y = attn/denom + x; batch-norm stats; sqrt on scalar."""
            recip = smalls.tile([128, 1], F32, tag="recip")
            nc.vector.reciprocal(recip[:], denom[:])
            y = work.tile([128, GT], F32, tag="y")
            nc.vector.scalar_tensor_tensor(
                y[:], ps_a[:], recip[:], x_t[:], op0=OP.mult, op1=OP.add
            )
            stats = smalls.tile([128, 6], F32, tag="stats")
            nc.vector.bn_stats(stats[:], y[:])
            mv = smalls.tile([128, 2], F32, tag="mv")
            nc.vector.bn_aggr(mv[:], stats[:])
            std = smalls.tile([128, 1], F32, tag="std")
            nc.scalar.activation(std[:], mv[:, 1:2], AF.Sqrt, bias=eps_b[:], scale=1.0)
            return y, mv, std

        def epi_norm_b(y, mv, std):
            rstd = smalls.tile([128, 1], F32, tag="rstd")
            nc.vector.reciprocal(rstd[:], std[:])
            o_t = work.tile([128, GT], F32, tag="o_t")
            nc.vector.tensor_scalar(
                o_t[:], y[:], mv[:, 0:1], rstd[:], op0=OP.subtract, op1=OP.mult
            )
            return o_t

        # slot-0 epilogue interleaved with slot-1 trees so the in-order
        # vector queue never blocks on a not-yet-satisfied dependency:
        #   ttr needs ps_s0 (ready right after s0c4's matmul)
        #   recip/y/bn need exp0 + attn psum (ready during s1c1's tree)
        #   rstd/TS need sqrt0 (ready during s1c2's tree)
        emit_tree(1, 1, ps_s1, W1, NCH1, mm_inline=False)
        flush_mms()
        for c in (2, 3, 4):
            emit_tree(1, c, ps_s1, W1, NCH1)
        w0, denom0 = epi_scores(0, W0, ps_s0, bits_sb[0])
        ps_a0 = epi_attn(w0, NKC0)
        emit_tree(1, 7, ps_s1, W1, NCH1)
        y0, mv0, std0 = epi_norm_a(ps_a0, denom0, x_t0)
        o_t0 = epi_norm_b(y0, mv0, std0)
        sdma.dma_start(out_d[0:128, :], o_t0[:])

        # slot-1 epilogue (the tail)
        w1, denom1 = epi_scores(1, W1, ps_s1, bits_sb[1])
        ps_a1 = epi_attn(w1, NKC1)
        y1, mv1, std1 = epi_norm_a(ps_a1, denom1, x_t1)
        o_t1 = epi_norm_b(y1, mv1, std1)
        sdma.dma_start(out_d[128:256, :], o_t1[:])

    nc.compile()
    return nc


def _get_nc():
    global _CACHED_NC
    if _CACHED_NC is None:
        _CACHED_NC = _build_program()
    return _CACHED_NC


def _tiles_for(h: int) -> tuple[int, int]:
    """Row-tile indices (slot0, slot1) handled by pair-half h."""
    return 3 - h, h


def kernel(x, x_query, context_key, context_value, gamma, beta):
    global LAST_RESULTS
    x = np.asarray(x, np.float32)
    x_query = np.asarray(x_query, np.float32)
    context_key = np.asarray(context_key, np.float32)
    context_value = np.asarray(context_value, np.float32)
    gamma = np.asarray(gamma, np.float32)
    beta = np.asarray(beta, np.float32)

    nc = _get_nc()
    p_idx = np.arange(128, dtype=np.float32)
    in_maps = []
    for c in range(NCORES):
        b, h = c // 2, c % 2
        t0, t1 = _tiles_for(h)
        r0 = slice(t0 * 128, (t0 + 1) * 128)
        r1 = slice(t1 * 128, (t1 + 1) * 128)
        thr = np.empty((128, 2), np.float32)
        thr[:, 0] = t0 * 128 + p_idx
        thr[:, 1] = t1 * 128 + p_idx
        in_maps.append({
            "xq": np.concatenate([x_query[b, r0], x_query[b, r1]]),
            "ck": np.ascontiguousarray(context_key[b, h * K_LOC:(h + 1) * K_LOC]),
            "cv": np.ascontiguousarray(context_value[b]),
            "xres": np.concatenate([x[b, r0], x[b, r1]]),
            "thr": thr,
        })

    if os.environ.get("KERNEL_TRACE") or os.environ.get("BASS_TRACE"):
        _ensure_trace_hook()
    res = run_bass_kernel_spmd(
        nc,
        in_maps,
        core_ids=list(range(NCORES)),
        trace=bool(os.environ.get("KERNEL_TRACE")),
    )
    LAST_RESULTS = res

    out = np.empty((B, L, GT), np.float32)
    for c, r in enumerate(res.results):
        b, h = c // 2, c % 2
        t0, t1 = _tiles_for(h)
        out[b, t0 * 128:(t0 + 1) * 128] = r["out"][0:128]
        out[b, t1 * 128:(t1 + 1) * 128] = r["out"][128:256]
    # LN affine (gamma/beta broadcast over the last axis) applied on host.
    out = out * gamma + beta
    return out.astype(np.float32)
